# revision 12
# baseline (speedup 1.0000x reference)
"""Trainium2 Bass kernel for nn_CustomAttention (relative-position attention).

Self-contained: hardcodes shapes from the problem spec.
  B=4, S=1024, E=1024, H=16 heads, D=64, MAX_REL=64.

Sharding: core c handles batch b=c//2 and head-half c%2 (8 heads), i.e. the
column slice [512*half:512*(half+1)] of Wq/Wk/Wv and row slice of Wo. Each
core returns its 8 heads of attn_weights and a partial (pre-bias) out; the
host sums the two per-batch partials and concatenates heads.

attn_mask is all-ones per the spec (fill "ones"); if a mask with zeros is
ever passed, kernel() falls back to a numpy reference implementation.
"""
import os
import sys
import contextlib

import numpy as np
import ml_dtypes

sys.path.insert(0, "/opt/trn_rl_repo")

import concourse.bass as bass
import concourse.tile as tile
from concourse import mybir
from concourse.bass_utils import run_bass_kernel_spmd

B, S, E = 4, 1024, 1024
H, D, MAX_REL = 16, 64, 64
HC = 8          # heads per core
W = 512         # projection width per core
P = 128
F32 = mybir.dt.float32
F32R = mybir.dt.float32r
BF16 = mybir.dt.bfloat16

LAST_RESULTS = None  # BassKernelResults of the latest run (for test harness)
LAST_RUN_S = None    # wall seconds of the latest run_bass_kernel_spmd call


def _split_multi_waits(nc, max_waits=1):
    """This walrus build rejects >1 sync wait per instruction. Move extras
    onto same-engine NoOps placed immediately before the instruction."""
    n = 0
    for func in nc.m.functions:
        for blk in func.blocks:
            new_insts = []
            for inst in blk.instructions:
                si = inst.sync_info
                waits = list(si.on_wait) if si is not None and si.on_wait else []
                if len(waits) > max_waits:
                    extra, keep = waits[:-max_waits], waits[-max_waits:]
                    for j, w in enumerate(extra):
                        new_insts.append(mybir.InstNoOp(
                            name=f"{inst.name}-ws{j}", engine=inst.engine,
                            ins=[], outs=[],
                            sync_info=mybir.SyncInfo(on_wait=[w], on_update=[])))
                        n += 1
                    si.on_wait = keep
                new_insts.append(inst)
            try:
                blk.instructions = new_insts
            except Exception:
                blk.instructions.clear()
                blk.instructions.extend(new_insts)
    return n


def build_nc():
    nc = bass.Bass(target_bir_lowering=False, debug=False)

    # ---- I/O ----
    xqT_d = nc.dram_tensor("xqT", [E, S], F32, kind="ExternalInput")
    xkT_d = nc.dram_tensor("xkT", [E, S], F32, kind="ExternalInput")
    xvT_d = nc.dram_tensor("xvT", [E, S], F32, kind="ExternalInput")
    Wq_d = nc.dram_tensor("Wq", [E, W], F32, kind="ExternalInput")
    Wk_d = nc.dram_tensor("Wk", [E, W], F32, kind="ExternalInput")
    Wv_d = nc.dram_tensor("Wv", [E, W], F32, kind="ExternalInput")
    Wo_d = nc.dram_tensor("Wo", [W, E], F32, kind="ExternalInput")
    bq_d = nc.dram_tensor("bq", [P, 4], F32, kind="ExternalInput")   # [p, t] stripe-major
    bk_d = nc.dram_tensor("bk", [P, 4], F32, kind="ExternalInput")
    bv_d = nc.dram_tensor("bv", [1, W], F32, kind="ExternalInput")
    ervT_d = nc.dram_tensor("ervT", [P, 256], F32, kind="ExternalInput")  # E[::-1].T padded, dup on both 64-halves
    attn_d = nc.dram_tensor("attn_w", [HC, S, S], F32, kind="ExternalOutput")
    outp_d = nc.dram_tensor("out_part", [S, E], F32, kind="ExternalOutput")

    scale = 1.0 / 8.0  # 1/sqrt(D)

    with tile.TileContext(nc) as tc:
        with contextlib.ExitStack() as ctx:
            singles = ctx.enter_context(tc.tile_pool(name="singles", bufs=1))
            xpool = ctx.enter_context(tc.tile_pool(name="xpool", bufs=1))
            wpool = ctx.enter_context(tc.tile_pool(name="wpool", bufs=1))
            proj = ctx.enter_context(tc.tile_pool(name="proj", bufs=1))
            apool = ctx.enter_context(tc.tile_pool(name="apool", bufs=2))
            upool = ctx.enter_context(tc.tile_pool(name="upool", bufs=2))
            dram = ctx.enter_context(tc.tile_pool(name="dram", bufs=4, space="DRAM"))
            ps_s_pool = ctx.enter_context(tc.tile_pool(name="ps_s", bufs=2, space="PSUM"))
            ps_p_pool = ctx.enter_context(tc.tile_pool(name="ps_p", bufs=2, space="PSUM"))
            ps_o_pool = ctx.enter_context(tc.tile_pool(name="ps_o", bufs=1, space="PSUM"))

            # ---------- constants ----------
            erv_sb = singles.tile([P, 256], F32R)
            nc.sync.dma_start(erv_sb[:], ervT_d[:].bitcast(F32R))
            bq_sb = singles.tile([P, 4], F32)
            bk_sb = singles.tile([P, 4], F32)
            nc.sync.dma_start(bq_sb[:], bq_d[:])
            nc.sync.dma_start(bk_sb[:], bk_d[:])
            bv_ap = bv_d[:]
            bvb_sb = singles.tile([P, W], F32)
            nc.sync.dma_start(bvb_sb[:], bass.AP(
                tensor=bv_ap.tensor, offset=bv_ap.offset, ap=[[0, P], [1, W]]))

            # persistent activations
            qT_sb = proj.tile([P, 4, S], F32R)   # [w%128, w//128, s]
            kT_sb = proj.tile([P, 4, S], F32R)   # scaled by 1/8, bias added

            # ---------- projections ----------
            def project(xT_d, W_d, out_kind):
                x_sb = xpool.tile([P, 8, S], F32R, tag="xbuf")
                nc.sync.dma_start(x_sb[:], xT_d[:].rearrange("(o p) s -> p o s", p=P).bitcast(F32R))
                w_sb = wpool.tile([P, 8, W], F32R, tag="wbuf")
                nc.sync.dma_start(w_sb[:], W_d[:].rearrange("(o p) w -> p o w", p=P).bitcast(F32R))
                if out_kind in ("q", "k"):
                    dst = qT_sb if out_kind == "q" else kT_sb
                    bias = bq_sb if out_kind == "q" else bk_sb
                    for t in range(4):
                        for n in range(2):
                            ps = ps_s_pool.tile([P, 512], F32, tag="ps_sc")
                            for k in range(8):
                                nc.tensor.matmul(
                                    ps[:], w_sb[:, k, 128 * t:128 * t + 128],
                                    x_sb[:, k, 512 * n:512 * n + 512],
                                    start=(k == 0), stop=(k == 7))
                            dstv = dst[:, t, 512 * n:512 * n + 512]
                            if out_kind == "q":
                                nc.vector.tensor_scalar_add(
                                    out=dstv, in0=ps[:], scalar1=bias[:, t:t + 1])
                            else:
                                nc.vector.tensor_scalar(
                                    out=dstv, in0=ps[:], scalar1=bias[:, t:t + 1],
                                    scalar2=scale, op0=mybir.AluOpType.add,
                                    op1=mybir.AluOpType.mult)
                else:  # v: natural layout [s, w] in stripes
                    for a in range(8):
                        ps = ps_s_pool.tile([P, 512], F32, tag="ps_sc")
                        for k in range(8):
                            nc.tensor.matmul(
                                ps[:], x_sb[:, k, 128 * a:128 * a + 128],
                                w_sb[:, k, :], start=(k == 0), stop=(k == 7))
                        nc.vector.scalar_tensor_tensor(
                            out=v16_sb[:, a, :], in0=ps[:], scalar=1.0,
                            in1=bvb_sb[:], op0=mybir.AluOpType.mult,
                            op1=mybir.AluOpType.add)

            v16_sb = proj.tile([P, 8, W], BF16)  # [s_part, s_stripe, w]
            project(xqT_d, Wq_d, "q")
            project(xkT_d, Wk_d, "k")
            project(xvT_d, Wv_d, "v")

            Wo_sb = singles.tile([P, 4, E], F32R)
            nc.sync.dma_start(Wo_sb[:], Wo_d[:].rearrange("(o p) e -> p o e", p=P).bitcast(F32R))

            outcat_sb = proj.tile([P, 4, S], F32R)  # outT stripes [hd%128, hd//128, s]

            # ---------- attention ----------
            for h in range(HC):
                t, base = h // 2, 64 * (h % 2)
                qT_h = qT_sb[base:base + 64, t, :]
                kT_h = kT_sb[base:base + 64, t, :]
                UT = upool.tile([P, 8, S], BF16, tag="UT")     # [j_part, j_stripe? no: [jj, c?]...
                # UT[jj, c, i]: block c covers j=128c..128c+128; i full 0..1024
                sums_h = apool.tile([P, 8], F32, tag="sums")
                if h % 2 == 0:
                    ps_o = ps_o_pool.tile([P, S], F32, tag="ps_o")
                    rrep = apool.tile([P, S], F32, tag="rrep")

                for a in range(8):
                    ps_sc = ps_s_pool.tile([P, S], F32, tag="ps_sc")
                    for n in range(2):
                        nc.tensor.matmul(
                            ps_sc[:, 512 * n:512 * n + 512],
                            qT_h[:, 128 * a:128 * a + 128],
                            kT_h[:, 512 * n:512 * n + 512],
                            start=True, stop=True)
                    ps_pr = ps_p_pool.tile([P, 256], F32, tag="ps_pr")
                    nc.tensor.matmul(
                        ps_pr[:], qT_h[:, 128 * a:128 * a + 128],
                        erv_sb[base:base + 64, :], start=True, stop=True)

                    # prx: row ii = [cL*128, pr[ii,0..129], cR*127]
                    prx = apool.tile([P, 384], F32, tag="prx")
                    nc.vector.tensor_copy(prx[:, 128:257], ps_pr[:, 0:129])
                    nc.vector.tensor_scalar(
                        out=prx[:, 0:128], in0=prx[:, 128:256], scalar1=0.0,
                        scalar2=prx[:, 128:129], op0=mybir.AluOpType.mult,
                        op1=mybir.AluOpType.add)
                    nc.vector.tensor_scalar(
                        out=prx[:, 257:384], in0=prx[:, 129:256], scalar1=0.0,
                        scalar2=prx[:, 256:257], op0=mybir.AluOpType.mult,
                        op1=mybir.AluOpType.add)
                    zp = dram.tile([P, 384], F32, tag="zp")
                    nc.sync.dma_start(zp[:], prx[:])
                    relw = apool.tile([P, 256], F32, tag="relw")
                    zp_ap = zp[:]
                    nc.sync.dma_start(relw[:], bass.AP(
                        tensor=zp_ap.tensor, offset=zp_ap.offset + 128,
                        ap=[[383, P], [1, 256]]))

                    wL = max(0, 128 * a - 64)
                    wR = min(S, 128 * a + 192)
                    woff = wL - (128 * a - 64)
                    nc.vector.tensor_tensor(
                        out=ps_sc[:, wL:wR], in0=ps_sc[:, wL:wR],
                        in1=relw[:, woff:woff + (wR - wL)], op=mybir.AluOpType.add)

                    cL = prx[:, 128:129]
                    cR = prx[:, 256:257]
                    # pass 1: U16 = exp(scores [+ clip bias]), accum row sums
                    U16 = upool.tile([P, S], BF16, tag="U16")
                    s3 = apool.tile([P, 3], F32, tag="s3")
                    if wL > 0:
                        nc.scalar.activation(U16[:, 0:wL], ps_sc[:, 0:wL],
                                             mybir.ActivationFunctionType.Exp,
                                             bias=cL, accum_out=s3[:, 0:1])
                    else:
                        nc.vector.memset(s3[:, 0:1], 0.0)
                    nc.scalar.activation(U16[:, wL:wR], ps_sc[:, wL:wR],
                                         mybir.ActivationFunctionType.Exp,
                                         bias=0.0, accum_out=s3[:, 1:2])
                    if wR < S:
                        nc.scalar.activation(U16[:, wR:S], ps_sc[:, wR:S],
                                             mybir.ActivationFunctionType.Exp,
                                             bias=cR, accum_out=s3[:, 2:3])
                    else:
                        nc.vector.memset(s3[:, 2:3], 0.0)
                    nc.vector.tensor_reduce(
                        out=sums_h[:, a:a + 1], in_=s3[:],
                        axis=mybir.AxisListType.X, op=mybir.AluOpType.add)

                    # pass 2: attn = exp(scores + bias - ln(sum)) -> fp32 output
                    lns = apool.tile([P, 1], F32, tag="lns")
                    nc.scalar.activation(lns[:], sums_h[:, a:a + 1],
                                         mybir.ActivationFunctionType.Ln)
                    nls = apool.tile([P, 4], F32, tag="nls")
                    nc.vector.tensor_scalar_mul(out=nls[:, 0:1], in0=lns[:], scalar1=-1.0)
                    nc.vector.scalar_tensor_tensor(
                        out=nls[:, 1:2], in0=cL, scalar=1.0, in1=nls[:, 0:1],
                        op0=mybir.AluOpType.mult, op1=mybir.AluOpType.add)
                    nc.vector.scalar_tensor_tensor(
                        out=nls[:, 2:3], in0=cR, scalar=1.0, in1=nls[:, 0:1],
                        op0=mybir.AluOpType.mult, op1=mybir.AluOpType.add)
                    attn_sb = upool.tile([P, S], F32, tag="attn")
                    if wL > 0:
                        nc.scalar.activation(attn_sb[:, 0:wL], ps_sc[:, 0:wL],
                                             mybir.ActivationFunctionType.Exp,
                                             bias=nls[:, 1:2])
                    nc.scalar.activation(attn_sb[:, wL:wR], ps_sc[:, wL:wR],
                                         mybir.ActivationFunctionType.Exp,
                                         bias=nls[:, 0:1])
                    if wR < S:
                        nc.scalar.activation(attn_sb[:, wR:S], ps_sc[:, wR:S],
                                             mybir.ActivationFunctionType.Exp,
                                             bias=nls[:, 2:3])
                    nc.sync.dma_start(attn_d[h, 128 * a:128 * a + 128, :], attn_sb[:])

                    # transpose U16 into UT blocks (XBAR, bf16)
                    for c in range(8):
                        nc.scalar.dma_start_transpose(
                            UT[:, c, 128 * a:128 * a + 128],
                            U16[:, 128 * c:128 * c + 128])

                # out matmuls: outT_h[d, i] = sum_j v[j, d] * U^T[j, i]
                for n in range(2):
                    for c in range(8):
                        nc.tensor.matmul(
                            ps_o[base:base + 64, 512 * n:512 * n + 512],
                            v16_sb[:, c, 64 * h:64 * h + 64],
                            UT[:, c, 512 * n:512 * n + 512],
                            start=(c == 0), stop=(c == 7))

                # reciprocal sums -> broadcast [64, S] rows of rrep
                rec = apool.tile([P, 8], F32, tag="rec")
                nc.vector.reciprocal(rec[:], sums_h[:])
                rd = dram.tile([1, S], F32, tag="rd")
                rd_ap = rd[:]
                nc.sync.dma_start(
                    bass.AP(tensor=rd_ap.tensor, offset=rd_ap.offset,
                            ap=[[1, P], [P, 8]]), rec[:])
                nc.sync.dma_start(
                    rrep[base:base + 64, :],
                    bass.AP(tensor=rd_ap.tensor, offset=rd_ap.offset,
                            ap=[[0, 64], [1, S]]))

                if h % 2 == 1:
                    nc.vector.tensor_tensor(
                        out=outcat_sb[:, t, :], in0=ps_o[:],
                        in1=rrep[:], op=mybir.AluOpType.mult)

            # ---------- output projection ----------
            for a in range(8):
                for n in range(2):
                    ps = ps_s_pool.tile([P, 512], F32, tag="ps_sc")
                    for t in range(4):
                        nc.tensor.matmul(
                            ps[:], outcat_sb[:, t, 128 * a:128 * a + 128],
                            Wo_sb[:, t, 512 * n:512 * n + 512],
                            start=(t == 0), stop=(t == 3))
                    osb = apool.tile([P, 512], F32, tag="osb")
                    nc.vector.tensor_copy(osb[:], ps[:])
                    nc.sync.dma_start(
                        outp_d[128 * a:128 * a + 128, 512 * n:512 * n + 512], osb[:])

    _split_multi_waits(nc, max_waits=1)
    return nc


_NC_CACHE = None


def _numpy_reference(query, key, value, attn_mask, Wq, bq, Wk, bk, Wv, bv, Wo, bo,
                     rel_pos_embed):
    q = (query @ Wq + bq).reshape(B, S, H, D).transpose(0, 2, 1, 3)
    k = (key @ Wk + bk).reshape(B, S, H, D).transpose(0, 2, 1, 3)
    v = (value @ Wv + bv).reshape(B, S, H, D).transpose(0, 2, 1, 3)
    scores = np.einsum("bhqd,bhkd->bhqk", q, k) / np.sqrt(D).astype(np.float32)
    rng = np.arange(S)
    rmat = np.clip(rng[:, None] - rng[None, :], -MAX_REL, MAX_REL) + MAX_REL
    rel = rel_pos_embed[rmat]
    scores = scores + np.einsum("bhid,ijd->bhij", q, rel)
    scores = np.where(attn_mask == 0, -np.inf, scores)
    m = scores.max(-1, keepdims=True)
    e = np.exp(scores - m)
    aw = e / e.sum(-1, keepdims=True)
    out = np.einsum("bhqk,bhkd->bhqd", aw, v)
    out = out.transpose(0, 2, 1, 3).reshape(B, S, E)
    return (out @ Wo + bo).astype(np.float32), aw.astype(np.float32)


def kernel(query, key, value, attn_mask, Wq, bq, Wk, bk, Wv, bv, Wo, bo,
           rel_pos_embed, **_ignored):
    global _NC_CACHE, LAST_RESULTS
    query = np.asarray(query, np.float32)
    key = np.asarray(key, np.float32)
    value = np.asarray(value, np.float32)
    attn_mask = np.asarray(attn_mask)
    Wq, bq = np.asarray(Wq, np.float32), np.asarray(bq, np.float32)
    Wk, bk = np.asarray(Wk, np.float32), np.asarray(bk, np.float32)
    Wv, bv = np.asarray(Wv, np.float32), np.asarray(bv, np.float32)
    Wo, bo = np.asarray(Wo, np.float32), np.asarray(bo, np.float32)
    rel_pos_embed = np.asarray(rel_pos_embed, np.float32)

    if not np.all(attn_mask != 0):
        return _numpy_reference(query, key, value, attn_mask, Wq, bq, Wk, bk,
                                Wv, bv, Wo, bo, rel_pos_embed)

    if _NC_CACHE is None:
        _NC_CACHE = build_nc()
    nc = _NC_CACHE

    # E[::-1].T padded to [128, 256], duplicated across both 64-partition halves
    erv = np.zeros((P, 256), np.float32)
    ervT = rel_pos_embed[::-1].T.astype(np.float32)      # [64, 129]
    erv[0:64, 0:129] = ervT
    erv[64:128, 0:129] = ervT

    in_maps = []
    for c in range(8):
        b, half = c // 2, c % 2
        cols = slice(512 * half, 512 * half + 512)
        in_maps.append({
            "xqT": np.ascontiguousarray(query[b].T),
            "xkT": np.ascontiguousarray(key[b].T),
            "xvT": np.ascontiguousarray(value[b].T),
            "Wq": np.ascontiguousarray(Wq[:, cols]),
            "Wk": np.ascontiguousarray(Wk[:, cols]),
            "Wv": np.ascontiguousarray(Wv[:, cols]),
            "Wo": np.ascontiguousarray(Wo[cols, :]),
            "bq": np.ascontiguousarray(bq[cols].reshape(4, P).T),
            "bk": np.ascontiguousarray(bk[cols].reshape(4, P).T),
            "bv": np.ascontiguousarray(bv[cols].reshape(1, W)),
            "ervT": erv,
        })

    trace = os.environ.get("KERNEL_TRACE", "0") == "1"
    import time as _time
    t0 = _time.monotonic()
    try:
        res = run_bass_kernel_spmd(nc, in_maps, core_ids=list(range(8)), trace=trace)
    except ModuleNotFoundError:
        res = run_bass_kernel_spmd(nc, in_maps, core_ids=list(range(8)), trace=False)
    global LAST_RUN_S
    LAST_RUN_S = _time.monotonic() - t0
    LAST_RESULTS = res

    attn_full = np.empty((B, H, S, S), np.float32)
    out_full = np.empty((B, S, E), np.float32)
    for b in range(B):
        r0 = res.results[2 * b]
        r1 = res.results[2 * b + 1]
        attn_full[b, 0:8] = r0["attn_w"]
        attn_full[b, 8:16] = r1["attn_w"]
        out_full[b] = r0["out_part"] + r1["out_part"] + bo
    return out_full, attn_full
